# revision 5
# baseline (speedup 1.0000x reference)
# Multi-head self-attention (B=2, S=4096, D=512, H=8) on 8 NeuronCores.
#
# Sharding: core c -> batch b = c//4, head-pair hp = c%4 (heads 2hp, 2hp+1,
# i.e. channels [128*hp, 128*hp+128) of the QKV projection space).
# Host pre-slices/transposes weights + x per core (cast fp16 for the PE);
# device does all matmuls (QKV projections, flash-attention with fused
# softmax, W_O row-slice projection); host sums the 4 per-core W_O partials
# per batch (the "all-reduce") and transposes back.
#
# Per-core device kernel (matmul operands fp16, accumulation fp32 PSUM):
#   qtz_h/ktz_h [128, 4096]: Q^T/K^T per head, dk on a 64-partition band,
#     zero elsewhere -> every scores matmul is full-K (128,128) mode.
#   scoresT[kpos, q] = K Q^T chunkwise -> ACT exp(x/8) straight from PSUM
#   AV with a ones-column appended to V -> denominator for free
#   softmax division via DVE reciprocal + gpsimd partition_broadcast.

import numpy as np

B, S, D, H, DK = 2, 4096, 512, 8, 64
P = 128          # partition tile
NQ = 512         # matmul moving free dim (one fp32 PSUM bank)
QCH = 1024       # q-chunk (2 x NQ) => one [128,1024] exp per kpos-chunk
NKC = S // P     # kpos chunks (32)
NST = S // NQ    # s-tiles of 512 (8)
NDC = D // P     # d chunks (4)
NQC = S // QCH   # q chunks (4)

TRACE = False            # test.py sets True to get exec_time_ns + perfetto
TMPDIR = None            # optional trace output dir
LAST_RESULTS = None      # BassKernelResults of the last run (for test.py)

_CACHE = {}


def _build_nc():
    import concourse.bass as bass  # noqa: F401
    import concourse.mybir as mybir
    import concourse.tile as tile
    from concourse import bacc
    from concourse.masks import make_identity

    f32 = mybir.dt.float32
    f16 = mybir.dt.float16
    Act = mybir.ActivationFunctionType

    nc = bacc.Bacc("TRN2", target_bir_lowering=False, debug=False, num_devices=8)

    xT = nc.dram_tensor("xT", [D, S], f16, kind="ExternalInput")
    wqT = nc.dram_tensor("wqT", [D, P], f16, kind="ExternalInput")
    wkT = nc.dram_tensor("wkT", [D, P], f16, kind="ExternalInput")
    wvT = nc.dram_tensor("wvT", [D, P], f16, kind="ExternalInput")
    woT0 = nc.dram_tensor("woT0", [DK, D], f16, kind="ExternalInput")
    woT1 = nc.dram_tensor("woT1", [DK, D], f16, kind="ExternalInput")
    yT = nc.dram_tensor("yT", [D, S], f32, kind="ExternalOutput")

    with tile.TileContext(nc) as tc:
        with tc.tile_pool(name="pers", bufs=1) as pers:
            # Q^T / K^T per head, padded to K=128 with a zero band.
            qtz = [pers.tile([P, S], f16, tag=f"qtz{h}", name=f"qtz{h}")
                   for h in range(2)]
            ktz = [pers.tile([P, S], f16, tag=f"ktz{h}", name=f"ktz{h}")
                   for h in range(2)]
            # V per head, [kpos, dk] chunks with a ones col at 64: [128, 33*65]
            vb = [pers.tile([P, NKC * (DK + 1)], f16, tag=f"vb{h}", name=f"vb{h}")
                  for h in range(2)]

            nc.gpsimd.memset(qtz[0][DK:P, :], 0.0)
            nc.gpsimd.memset(ktz[0][DK:P, :], 0.0)
            nc.gpsimd.memset(qtz[1][0:DK, :], 0.0)
            nc.gpsimd.memset(ktz[1][0:DK, :], 0.0)
            nc.gpsimd.memset(vb[0][:, :], 1.0)
            nc.gpsimd.memset(vb[1][:, :], 1.0)

            # ---------------- phase 1: load x, QKV projections, build V ----
            with (
                tc.tile_pool(name="ph1", bufs=1) as ph1,
                tc.tile_pool(name="ph1ps", bufs=1, space="PSUM") as ph1ps,
            ):
                xts = [ph1.tile([P, S], f16, tag=f"xt{dc}", name=f"xt{dc}")
                       for dc in range(NDC)]
                for dc in range(NDC):
                    for blk in range(4):
                        sl = slice(blk * 1024, (blk + 1) * 1024)
                        nc.sync.dma_start(xts[dc][:, sl], xT[dc * P:(dc + 1) * P, sl])

                wsb = {}
                for name, dram in (("q", wqT), ("k", wkT), ("v", wvT)):
                    w = ph1.tile([P, NDC * P], f16, tag=f"w{name}", name=f"w{name}")
                    for dc in range(NDC):
                        nc.sync.dma_start(
                            w[:, dc * P:(dc + 1) * P], dram[dc * P:(dc + 1) * P, :]
                        )
                    wsb[name] = w

                vt = ph1.tile([P, S], f32, tag="vt")
                ident = ph1.tile([P, P], f32, tag="ident")
                make_identity(nc, ident[:, :])

                for name in ("q", "k", "v"):
                    w = wsb[name]
                    for st in range(NST):
                        ps = ph1ps.tile([P, NQ], f32, tag="qkvps", bufs=3)
                        for dc in range(NDC):
                            nc.tensor.matmul(
                                ps[:, :],
                                w[:, dc * P:(dc + 1) * P],
                                xts[dc][:, st * NQ:(st + 1) * NQ],
                                start=(dc == 0),
                                stop=(dc == NDC - 1),
                            )
                        sl = slice(st * NQ, (st + 1) * NQ)
                        if name == "v":
                            nc.vector.tensor_copy(vt[:, sl], ps[:, :])
                        else:
                            dst = qtz if name == "q" else ktz
                            nc.vector.tensor_copy(dst[0][0:DK, sl], ps[0:DK, :])
                            nc.vector.tensor_copy(dst[1][DK:P, sl], ps[DK:P, :])

                # transpose V^T -> per-head V chunks [kpos, dk] (+ones col)
                for ch in range(NKC):
                    tp = ph1ps.tile([P, P], f32, tag="trps", bufs=2)
                    nc.tensor.transpose(
                        tp[:, :], vt[:, ch * P:(ch + 1) * P], ident[:, :]
                    )
                    c0 = ch * (DK + 1)
                    nc.vector.tensor_copy(vb[0][:, c0:c0 + DK], tp[:, 0:DK])
                    nc.vector.tensor_copy(vb[1][:, c0:c0 + DK], tp[:, DK:P])

            # ---------------- phase 2: flash attention + W_O ---------------
            with tc.tile_pool(name="att_o", bufs=1) as att_o:
                outt = [att_o.tile([DK, S], f16, tag=f"outt{h}", name=f"outt{h}")
                        for h in range(2)]
                wos = [att_o.tile([DK, D], f16, tag=f"wos{h}", name=f"wos{h}")
                       for h in range(2)]
                nc.sync.dma_start(wos[0][:, :], woT0[:, :])
                nc.sync.dma_start(wos[1][:, :], woT1[:, :])

                with (
                    tc.tile_pool(name="attps", bufs=1, space="PSUM") as attps,
                    tc.tile_pool(name="attsb", bufs=1) as attsb,
                ):
                    for qc in range(NQC):
                        av = {
                            (h, sub): attps.tile(
                                [DK + 1, NQ], f32, tag=f"av{h}{sub}",
                                name=f"av{h}{sub}",
                            )
                            for h in range(2)
                            for sub in range(2)
                        }
                        for k in range(NKC):
                            for h in range(2):
                                scp = attps.tile([P, QCH], f32, tag=f"sc{h}")
                                for sub in range(2):
                                    q0 = qc * QCH + sub * NQ
                                    nc.tensor.matmul(
                                        scp[:, sub * NQ:(sub + 1) * NQ],
                                        ktz[h][:, k * P:(k + 1) * P],
                                        qtz[h][:, q0:q0 + NQ],
                                        start=True,
                                        stop=True,
                                    )
                                ex = attsb.tile([P, QCH], f16, tag=f"ex{h}", bufs=3)
                                nc.scalar.activation(
                                    ex[:, :], scp[:, :], Act.Exp, scale=0.125
                                )
                                c0 = k * (DK + 1)
                                for sub in range(2):
                                    nc.tensor.matmul(
                                        av[h, sub][:, :],
                                        vb[h][:, c0:c0 + DK + 1],
                                        ex[:, sub * NQ:(sub + 1) * NQ],
                                        start=(k == 0),
                                        stop=(k == NKC - 1),
                                    )
                        # normalize: out = av[0:64] / av[64]
                        for h in range(2):
                            for sub in range(2):
                                a = av[h, sub]
                                dn = attsb.tile([P, NQ], f32, tag="dn", bufs=2)
                                nc.vector.tensor_copy(dn[DK:DK + 1, :], a[DK:DK + 1, :])
                                dn0 = attsb.tile([P, NQ], f32, tag="dn0", bufs=2)
                                nc.sync.dma_start(dn0[0:1, :], dn[DK:DK + 1, :])
                                rc = attsb.tile([P, NQ], f32, tag="rc", bufs=2)
                                nc.vector.reciprocal(rc[0:1, :], dn0[0:1, :])
                                rcb = attsb.tile([DK, NQ], f32, tag="rcb", bufs=2)
                                nc.gpsimd.partition_broadcast(
                                    rcb[:, :], rc[0:1, :], channels=DK
                                )
                                q0 = qc * QCH + sub * NQ
                                nc.vector.tensor_mul(
                                    outt[h][:, q0:q0 + NQ], a[0:DK, :], rcb[:, :]
                                )

                # W_O row-slice projection: yT[e, s] = sum_h wos_h.T @ outt_h
                with (
                    tc.tile_pool(name="wops", bufs=1, space="PSUM") as wops,
                    tc.tile_pool(name="wosb", bufs=1) as wosb,
                ):
                    for ec in range(NDC):
                        for st in range(NST):
                            yp = wops.tile([P, NQ], f32, tag="yp", bufs=4)
                            for h in range(2):
                                nc.tensor.matmul(
                                    yp[:, :],
                                    wos[h][:, ec * P:(ec + 1) * P],
                                    outt[h][:, st * NQ:(st + 1) * NQ],
                                    start=(h == 0),
                                    stop=(h == 1),
                                )
                            ys = wosb.tile([P, NQ], f32, tag="ys", bufs=4)
                            nc.vector.tensor_copy(ys[:, :], yp[:, :])
                            nc.sync.dma_start(
                                yT[ec * P:(ec + 1) * P, st * NQ:(st + 1) * NQ],
                                ys[:, :],
                            )

    nc.compile()
    return nc


def kernel(x, wq, wk, wv, wo):
    global LAST_RESULTS
    from concourse.bass_utils import run_bass_kernel_spmd

    if "nc" not in _CACHE:
        _CACHE["nc"] = _build_nc()
    nc = _CACHE["nc"]

    x = np.asarray(x, dtype=np.float32)
    wq = np.asarray(wq, dtype=np.float32)
    wk = np.asarray(wk, dtype=np.float32)
    wv = np.asarray(wv, dtype=np.float32)
    wo = np.asarray(wo, dtype=np.float32)

    in_maps = []
    for c in range(8):
        b, hp = divmod(c, 4)
        e0 = hp * P
        in_maps.append({
            "xT": np.ascontiguousarray(x[b].T.astype(np.float16)),
            "wqT": np.ascontiguousarray(wq[e0:e0 + P].T.astype(np.float16)),
            "wkT": np.ascontiguousarray(wk[e0:e0 + P].T.astype(np.float16)),
            "wvT": np.ascontiguousarray(wv[e0:e0 + P].T.astype(np.float16)),
            "woT0": np.ascontiguousarray(wo[:, e0:e0 + DK].T.astype(np.float16)),
            "woT1": np.ascontiguousarray(wo[:, e0 + DK:e0 + P].T.astype(np.float16)),
        })

    res = run_bass_kernel_spmd(
        nc, in_maps, core_ids=list(range(8)), trace=TRACE, tmpdir=TMPDIR
    )
    LAST_RESULTS = res

    y = np.zeros((B, S, D), dtype=np.float32)
    for c in range(8):
        y[c // 4] += res.results[c]["yT"].T
    return y


# revision 7
# speedup vs baseline: 1.1704x; 1.1704x over previous
# Multi-head self-attention (B=2, S=4096, D=512, H=8) on 8 NeuronCores.
#
# Sharding: core c -> batch b = c//4, head-pair hp = c%4 (heads 2hp, 2hp+1,
# i.e. channels [128*hp, 128*hp+128) of the QKV projection space).
# Host pre-slices/transposes weights + x per core (cast fp16 for the PE);
# device does all matmuls (QKV projections, flash-attention with fused
# softmax, W_O row-slice projection); host sums the 4 per-core W_O partials
# per batch (the "all-reduce") and transposes back.
#
# Per-core device kernel (matmul operands fp16, accumulation fp32 PSUM):
#   qtz_h/ktz_h [128, 4096]: Q^T/K^T per head, dk on a 64-partition band,
#     zero elsewhere -> every scores matmul is full-K (128,128) mode.
#   scoresT[kpos, q] = K Q^T chunkwise -> ACT exp(x/8) straight from PSUM
#   AV with a ones-column appended to V -> denominator for free
#   softmax division via DVE reciprocal + gpsimd partition_broadcast.

import numpy as np

B, S, D, H, DK = 2, 4096, 512, 8, 64
P = 128          # partition tile
NQ = 512         # matmul moving free dim (one fp32 PSUM bank)
QCH = 1024       # q-chunk (2 x NQ) => one [128,1024] exp per kpos-chunk
NKC = S // P     # kpos chunks (32)
NST = S // NQ    # s-tiles of 512 (8)
NDC = D // P     # d chunks (4)
NQC = S // QCH   # q chunks (4)

TRACE = False            # test.py sets True to get exec_time_ns + perfetto
TMPDIR = None            # optional trace output dir
LAST_RESULTS = None      # BassKernelResults of the last run (for test.py)

_CACHE = {}


def _build_nc():
    import concourse.bass as bass  # noqa: F401
    import concourse.mybir as mybir
    import concourse.tile as tile
    from concourse import bacc
    from concourse.masks import make_identity

    f32 = mybir.dt.float32
    f16 = mybir.dt.float16
    Act = mybir.ActivationFunctionType

    nc = bacc.Bacc("TRN2", target_bir_lowering=False, debug=False, num_devices=8)

    xT = nc.dram_tensor("xT", [D, S], f16, kind="ExternalInput")
    wqT = nc.dram_tensor("wqT", [D, P], f16, kind="ExternalInput")
    wkT = nc.dram_tensor("wkT", [D, P], f16, kind="ExternalInput")
    wvT = nc.dram_tensor("wvT", [D, P], f16, kind="ExternalInput")
    woT0 = nc.dram_tensor("woT0", [DK, D], f16, kind="ExternalInput")
    woT1 = nc.dram_tensor("woT1", [DK, D], f16, kind="ExternalInput")
    yT = nc.dram_tensor("yT", [D, S], f32, kind="ExternalOutput")

    with tile.TileContext(nc) as tc:
        with tc.tile_pool(name="pers", bufs=1) as pers:
            # Q^T / K^T per head, padded to K=128 with a zero band.
            qtz = [pers.tile([P, S], f16, tag=f"qtz{h}", name=f"qtz{h}")
                   for h in range(2)]
            ktz = [pers.tile([P, S], f16, tag=f"ktz{h}", name=f"ktz{h}")
                   for h in range(2)]
            # V per head, [kpos, dk] chunks with a ones col at 64: [128, 33*65]
            vb = [pers.tile([P, NKC * (DK + 1)], f16, tag=f"vb{h}", name=f"vb{h}")
                  for h in range(2)]

            nc.gpsimd.memset(qtz[0][DK:P, :], 0.0)
            nc.gpsimd.memset(ktz[0][DK:P, :], 0.0)
            nc.gpsimd.memset(qtz[1][0:DK, :], 0.0)
            nc.gpsimd.memset(ktz[1][0:DK, :], 0.0)
            nc.gpsimd.memset(vb[0][:, :], 1.0)
            nc.gpsimd.memset(vb[1][:, :], 1.0)

            # ---------------- phase 1: load x, QKV projections, build V ----
            with (
                tc.tile_pool(name="ph1", bufs=1) as ph1,
                tc.tile_pool(name="ph1ps", bufs=1, space="PSUM") as ph1ps,
            ):
                xts = [ph1.tile([P, S], f16, tag=f"xt{dc}", name=f"xt{dc}")
                       for dc in range(NDC)]
                wsb = {}
                for name, dram in (("q", wqT), ("k", wkT), ("v", wvT)):
                    w = ph1.tile([P, NDC * P], f16, tag=f"w{name}", name=f"w{name}")
                    for dc in range(NDC):
                        nc.sync.dma_start(
                            w[:, dc * P:(dc + 1) * P], dram[dc * P:(dc + 1) * P, :]
                        )
                    wsb[name] = w
                for blk in range(8):
                    sl = slice(blk * NQ, (blk + 1) * NQ)
                    for dc in range(NDC):
                        nc.sync.dma_start(xts[dc][:, sl], xT[dc * P:(dc + 1) * P, sl])

                vt = ph1.tile([P, S], f32, tag="vt")
                ident = ph1.tile([P, P], f32, tag="ident")
                make_identity(nc, ident[:, :])

                for name in ("v", "k", "q"):
                    w = wsb[name]
                    for st in range(NST):
                        ps = ph1ps.tile([P, NQ], f32, tag="qkvps", bufs=3)
                        for dc in range(NDC):
                            nc.tensor.matmul(
                                ps[:, :],
                                w[:, dc * P:(dc + 1) * P],
                                xts[dc][:, st * NQ:(st + 1) * NQ],
                                start=(dc == 0),
                                stop=(dc == NDC - 1),
                            )
                        sl = slice(st * NQ, (st + 1) * NQ)
                        if name == "v":
                            nc.vector.tensor_copy(vt[:, sl], ps[:, :])
                            # transpose this V window into per-head V chunks
                            # [kpos, dk] (+ones col) right away
                            for ch in range(4 * st, 4 * st + 4):
                                tp = ph1ps.tile([P, P], f32, tag="trps", bufs=2)
                                nc.tensor.transpose(
                                    tp[:, :], vt[:, ch * P:(ch + 1) * P],
                                    ident[:, :],
                                )
                                c0 = ch * (DK + 1)
                                nc.scalar.copy(vb[0][:, c0:c0 + DK], tp[:, 0:DK])
                                nc.vector.tensor_copy(
                                    vb[1][:, c0:c0 + DK], tp[:, DK:P]
                                )
                        else:
                            dst = qtz if name == "q" else ktz
                            nc.vector.tensor_copy(dst[0][0:DK, sl], ps[0:DK, :])
                            nc.vector.tensor_copy(dst[1][DK:P, sl], ps[DK:P, :])

            # ---------------- phase 2: flash attention + W_O ---------------
            with tc.tile_pool(name="att_o", bufs=1) as att_o:
                outt = [att_o.tile([DK, S], f16, tag=f"outt{h}", name=f"outt{h}")
                        for h in range(2)]
                wos = [att_o.tile([DK, D], f16, tag=f"wos{h}", name=f"wos{h}")
                       for h in range(2)]
                nc.sync.dma_start(wos[0][:, :], woT0[:, :])
                nc.sync.dma_start(wos[1][:, :], woT1[:, :])

                with (
                    tc.tile_pool(name="attps", bufs=1, space="PSUM") as attps,
                    tc.tile_pool(name="attsb", bufs=1) as attsb,
                ):
                    for qc in range(NQC):
                        av = {
                            (h, sub): attps.tile(
                                [DK + 1, NQ], f32, tag=f"av{h}{sub}",
                                name=f"av{h}{sub}",
                            )
                            for h in range(2)
                            for sub in range(2)
                        }
                        for k in range(NKC):
                            for h in range(2):
                                scp = attps.tile([P, QCH], f32, tag=f"sc{h}")
                                for sub in range(2):
                                    q0 = qc * QCH + sub * NQ
                                    nc.tensor.matmul(
                                        scp[:, sub * NQ:(sub + 1) * NQ],
                                        ktz[h][:, k * P:(k + 1) * P],
                                        qtz[h][:, q0:q0 + NQ],
                                        start=True,
                                        stop=True,
                                    )
                                ex = attsb.tile([P, QCH], f16, tag=f"ex{h}", bufs=3)
                                nc.scalar.activation(
                                    ex[:, :], scp[:, :], Act.Exp, scale=0.125
                                )
                                c0 = k * (DK + 1)
                                for sub in range(2):
                                    nc.tensor.matmul(
                                        av[h, sub][:, :],
                                        vb[h][:, c0:c0 + DK + 1],
                                        ex[:, sub * NQ:(sub + 1) * NQ],
                                        start=(k == 0),
                                        stop=(k == NKC - 1),
                                    )
                        # evacuate av psum fast (frees the bank), then do
                        # the softmax division off the critical path in SBUF
                        for h in range(2):
                            for sub in range(2):
                                a = av[h, sub]
                                raw = attsb.tile(
                                    [DK + 1, NQ], f32, tag=f"raw{h}{sub}",
                                    name=f"raw{h}{sub}", bufs=2,
                                )
                                nc.vector.tensor_copy(raw[:, :], a[:, :])
                                dn0 = attsb.tile([P, NQ], f32, tag="dn0", bufs=2)
                                nc.sync.dma_start(dn0[0:1, :], raw[DK:DK + 1, :])
                                rc = attsb.tile([P, NQ], f32, tag="rc", bufs=2)
                                nc.vector.reciprocal_approx_fast(
                                    rc[0:1, :], dn0[0:1, :]
                                )
                                rcb = attsb.tile([DK, NQ], f32, tag="rcb", bufs=2)
                                nc.gpsimd.partition_broadcast(
                                    rcb[:, :], rc[0:1, :], channels=DK
                                )
                                q0 = qc * QCH + sub * NQ
                                nc.vector.tensor_mul(
                                    outt[h][:, q0:q0 + NQ], raw[0:DK, :], rcb[:, :]
                                )

                # W_O row-slice projection: yT[e, s] = sum_h wos_h.T @ outt_h
                with (
                    tc.tile_pool(name="wops", bufs=1, space="PSUM") as wops,
                    tc.tile_pool(name="wosb", bufs=1) as wosb,
                ):
                    for ec in range(NDC):
                        for st in range(NST):
                            yp = wops.tile([P, NQ], f32, tag="yp", bufs=4)
                            for h in range(2):
                                nc.tensor.matmul(
                                    yp[:, :],
                                    wos[h][:, ec * P:(ec + 1) * P],
                                    outt[h][:, st * NQ:(st + 1) * NQ],
                                    start=(h == 0),
                                    stop=(h == 1),
                                )
                            ys = wosb.tile([P, NQ], f32, tag="ys", bufs=4)
                            nc.vector.tensor_copy(ys[:, :], yp[:, :])
                            nc.sync.dma_start(
                                yT[ec * P:(ec + 1) * P, st * NQ:(st + 1) * NQ],
                                ys[:, :],
                            )

    nc.compile()
    return nc


def kernel(x, wq, wk, wv, wo):
    global LAST_RESULTS
    from concourse.bass_utils import run_bass_kernel_spmd

    if "nc" not in _CACHE:
        _CACHE["nc"] = _build_nc()
    nc = _CACHE["nc"]

    x = np.asarray(x, dtype=np.float32)
    wq = np.asarray(wq, dtype=np.float32)
    wk = np.asarray(wk, dtype=np.float32)
    wv = np.asarray(wv, dtype=np.float32)
    wo = np.asarray(wo, dtype=np.float32)

    in_maps = []
    for c in range(8):
        b, hp = divmod(c, 4)
        e0 = hp * P
        in_maps.append({
            "xT": np.ascontiguousarray(x[b].T.astype(np.float16)),
            "wqT": np.ascontiguousarray(wq[e0:e0 + P].T.astype(np.float16)),
            "wkT": np.ascontiguousarray(wk[e0:e0 + P].T.astype(np.float16)),
            "wvT": np.ascontiguousarray(wv[e0:e0 + P].T.astype(np.float16)),
            "woT0": np.ascontiguousarray(wo[:, e0:e0 + DK].T.astype(np.float16)),
            "woT1": np.ascontiguousarray(wo[:, e0 + DK:e0 + P].T.astype(np.float16)),
        })

    res = run_bass_kernel_spmd(
        nc, in_maps, core_ids=list(range(8)), trace=TRACE, tmpdir=TMPDIR
    )
    LAST_RESULTS = res

    y = np.zeros((B, S, D), dtype=np.float32)
    for c in range(8):
        y[c // 4] += res.results[c]["yT"].T
    return y
